# revision 1
# baseline (speedup 1.0000x reference)
"""DGCNN encoder Bass kernel for Trainium2 (8 NeuronCores, data-parallel over batch).

Algorithm notes (per core, one point cloud of N=2048 points):
  Each edge conv  y_i = max_{j in knn20(i)} bn_lrelu(W @ [x_j - x_i; x_i])
  is reformulated with A = W[:, :C], Cm = W[:, C:] - W[:, :C]:
      y_i = lrelu( s * (max_j (A x_j) + Cm x_i) + beta )       (s = gamma/sqrt(1+eps) > 0)
  so no per-edge features are ever materialized: Z = X A''^T (N x O) is computed
  once, the kNN top-20 row gather of Z happens via indirect DMA, and the max
  over neighbors commutes with the per-point additive term and the monotone
  bn_lrelu.  kNN ranking uses D = X X^T - 0.5*|x_j|^2 (row-rank-equivalent to
  the reference's -dist^2), top-20 per row via 3 rounds of DVE max8 /
  max_index / match_replace (exact).
"""

import sys

if "/opt/trn_rl_repo" not in sys.path:
    sys.path.insert(0, "/opt/trn_rl_repo")

from contextlib import ExitStack

import numpy as np

import concourse.bass as bass
import concourse.mybir as mybir
from concourse import bacc
from concourse.bass import IndirectOffsetOnAxis
from concourse.bass_utils import run_bass_kernel_spmd
from concourse.masks import make_identity
from concourse.tile import TileContext, add_dep_helper

EPS = 1e-5
K = 20
NEG_BIG = -3.0e38
F32 = mybir.dt.float32
U32 = mybir.dt.uint32
X_AX = mybir.AxisListType.X
COPY = mybir.ActivationFunctionType.Copy
SQUARE = mybir.ActivationFunctionType.Square

# (C_in, O_out) per edge conv
CONVS = [(3, 64), (64, 64), (64, 128), (128, 256)]


def build_program(n_points=2048, max_conv=4, do_final=True, debug=False):
    """Build the single-core program (SPMD across 8 cores, batch-parallel)."""
    N = n_points
    T = N // 128  # row tiles
    JC = N // 512  # 512-wide column chunks of the distance matrix

    nc = bacc.Bacc(None, num_swdge_queues=4)

    pointsT = nc.declare_dram_parameter("pointsT", [3, N], F32, isOutput=False)
    acrhs = []
    acb = []
    for li, (C, O) in enumerate(CONVS):
        acrhs.append(
            nc.declare_dram_parameter(f"acrhs{li}", [C, 2 * O], F32, isOutput=False)
        )
        acb.append(
            nc.declare_dram_parameter(f"acb{li}", [1, 2 * O], F32, isOutput=False)
        )
    W5_SPLITS = [64, 64, 128, 128, 128]
    w5c = [
        nc.declare_dram_parameter(f"w5c{k}", [ck, 1024], F32, isOutput=False)
        for k, ck in enumerate(W5_SPLITS)
    ]
    b5row = nc.declare_dram_parameter("b5row", [1, 1024], F32, isOutput=False)
    out_t = nc.declare_dram_parameter("out", [1, 2 * 1024], F32, isOutput=True)
    if debug:
        dbg_ix = nc.declare_dram_parameter("dbg_ix", [N, 24], U32, isOutput=True)
        dbg_x1 = nc.declare_dram_parameter("dbg_x1", [64, N], F32, isOutputrue=True) if False else nc.declare_dram_parameter("dbg_x1", [64, N], F32, isOutput=True)
        dbg_hx = nc.declare_dram_parameter("dbg_hx", [1, N], F32, isOutput=True)
        dbg_z1 = nc.declare_dram_parameter("dbg_z1", [N, CONVS[0][1]], F32, isOutput=True)
        dbg_d0 = nc.declare_dram_parameter("dbg_d0", [128, N], F32, isOutput=True)
        dbg_g0 = nc.declare_dram_parameter("dbg_g0", [128, K * CONVS[0][1]], F32, isOutput=True)

    with ExitStack() as stack:
        tc = stack.enter_context(TileContext(nc))
        persist = stack.enter_context(tc.tile_pool(name="persist", bufs=1))
        dram = stack.enter_context(tc.tile_pool(name="dram", bufs=1, space="DRAM"))

        # ---- persistent SBUF state ----
        identity = persist.tile([128, 128], F32)
        make_identity(nc, identity[:])
        ones_row = persist.tile([1, N], F32)
        nc.vector.memset(ones_row[:], 1.0)
        onescol = persist.tile([128, 1], F32)
        nc.vector.memset(onescol[:], 1.0)
        halfx2neg = persist.tile([1, N], F32)

        xt0 = persist.tile([3, N], F32)
        nc.sync.dma_start(out=xt0[:], in_=pointsT[:])
        x1T = persist.tile([64, N], F32)
        x2T = persist.tile([64, N], F32)
        x3T = persist.tile([128, N], F32)
        x4Ta = persist.tile([128, N], F32)
        x4Tb = persist.tile([128, N], F32)

        acrhs_sb = []
        acb_sb = []
        for li, (C, O) in enumerate(CONVS):
            a = persist.tile([C, 2 * O], F32, tag=f"acrhs{li}")
            nc.sync.dma_start(out=a[:], in_=acrhs[li][:])
            acrhs_sb.append(a)
            b = persist.tile([1, 2 * O], F32, tag=f"acb{li}")
            nc.sync.dma_start(out=b[:], in_=acb[li][:])
            acb_sb.append(b)
        w5_sb = []
        for k, ck in enumerate(W5_SPLITS):
            w = persist.tile([ck, 1024], F32, tag=f"w5c{k}")
            nc.sync.dma_start(out=w[:], in_=w5c[k][:])
            w5_sb.append(w)
        b5_sb = persist.tile([1, 1024], F32)
        nc.sync.dma_start(out=b5_sb[:], in_=b5row[:])

        maxacc = persist.tile([128, 1024], F32)
        sumacc = persist.tile([128, 1024], F32)
        maxp = persist.tile([128, 8], F32)
        avgp = persist.tile([1, 1024], F32)

        z_dram = [
            dram.tile([N, O], F32, tag=f"z{li}", name=f"z{li}")
            for li, (_, O) in enumerate(CONVS)
        ]

        conv_in = [xt0, x1T, x2T, x3T]
        conv_out = [[(x1T, 0)], [(x2T, 0)], [(x3T, 0)], [(x4Ta, 0), (x4Tb, 128)]]

        # Final-stage pools are opened for the whole conv region so the W5
        # matmul + pooling accumulation for row-tile m can run interleaved
        # with conv4's pipeline as soon as x4T[:, mc] lands.
        pfp = stack.enter_context(tc.tile_pool(name="pf", bufs=1, space="PSUM"))
        fsp = stack.enter_context(tc.tile_pool(name="fs", bufs=1))
        if do_final:
            nc.vector.memset(maxacc[:], NEG_BIG)
            nc.vector.memset(sumacc[:], 0.0)
        xks = [x1T, x2T, x3T, x4Ta, x4Tb]

        def emit_final_tile(m):
            mc = slice(m * 128, (m + 1) * 128)
            pf = pfp.tile([128, 1024], F32, space="PSUM", tag="pf",
                          name=f"pf_{m}")
            for h in range(2):
                hc = slice(h * 512, (h + 1) * 512)
                for k, xk in enumerate(xks):
                    nc.tensor.matmul(
                        pf[:, hc], xk[:, mc], w5_sb[k][:, hc],
                        start=(k == 0), stop=False,
                    )
                nc.tensor.matmul(
                    pf[:, hc], ones_row[:, mc], b5_sb[:, hc],
                    start=False, stop=True,
                )
            fs = fsp.tile([128, 1024], F32, tag="fs", name=f"fs_{m}")
            nc.scalar.activation(out=fs[:], in_=pf[:, :], func=COPY, scale=0.2)
            nc.vector.tensor_tensor(
                out=fs[:], in0=pf[:, :], in1=fs[:], op=mybir.AluOpType.max
            )
            nc.vector.tensor_tensor(
                out=maxacc[:], in0=maxacc[:], in1=fs[:], op=mybir.AluOpType.max
            )
            nc.vector.tensor_add(out=sumacc[:], in0=sumacc[:], in1=fs[:])

        for li, (C, O) in enumerate(CONVS):
            if li >= max_conv:
                break
            xT = conv_in[li]
            with ExitStack() as cs:
                sqp = cs.enter_context(tc.tile_pool(name="sq", bufs=1))
                zstp = cs.enter_context(tc.tile_pool(name="zst", bufs=2))
                csbp = cs.enter_context(tc.tile_pool(name="csb", bufs=1))

                # ---- phase A: column norms -> halfx2neg = -0.5 * |x_j|^2 ----
                with tc.tile_pool(name="px2", bufs=1, space="PSUM") as px2p:
                    sq = sqp.tile([C, N], F32, tag="sq")
                    nc.scalar.activation(out=sq[:], in_=xT[:, :], func=SQUARE)
                    px2 = px2p.tile([1, N], F32, space="PSUM", tag="px2")
                    for j in range(JC):
                        jc = slice(j * 512, (j + 1) * 512)
                        nc.tensor.matmul(
                            px2[:, jc], onescol[0:C, :], sq[:, jc],
                            start=True, stop=True,
                        )
                    nc.scalar.activation(
                        out=halfx2neg[:, :], in_=px2[:, :], func=COPY, scale=-0.5
                    )
                    if debug and li == 0:
                        nc.sync.dma_start(out=dbg_hx[:], in_=halfx2neg[:, :])

                # ---- phase B: Z = X A''^T + 0*beta -> DRAM ; c = X Cm''^T + beta ----
                z_writes = []
                csb = csbp.tile([128, T, O], F32, tag="csb")
                with tc.tile_pool(name="pzc", bufs=2, space="PSUM") as pzcp:
                    for m in range(T):
                        mc = slice(m * 128, (m + 1) * 128)
                        pzc = pzcp.tile([128, 2 * O], F32, space="PSUM", tag="pzc")
                        nc.tensor.matmul(
                            pzc[:, :], xT[:, mc], acrhs_sb[li][:, :],
                            start=True, stop=False,
                        )
                        nc.tensor.matmul(
                            pzc[:, :], ones_row[:, mc], acb_sb[li][:, :],
                            start=False, stop=True,
                        )
                        zst = zstp.tile([128, O], F32, tag="zst")
                        nc.scalar.copy(out=zst[:], in_=pzc[:, 0:O])
                        nc.scalar.copy(out=csb[:, m, :], in_=pzc[:, O : 2 * O])
                        zw = nc.sync.dma_start(out=z_dram[li][mc, :], in_=zst[:])
                        z_writes.append(zw.ins)
                        if debug and li == 0:
                            nc.sync.dma_start(out=dbg_z1[mc, :], in_=zst[:])

                # ---- phase C: per row-tile D, top-20, gather, reduce ----
                with ExitStack() as ps:
                    pdp = ps.enter_context(
                        tc.tile_pool(name="pd", bufs=1, space="PSUM")
                    )
                    ptrp = ps.enter_context(
                        tc.tile_pool(name="ptr", bufs=2, space="PSUM")
                    )
                    dsbp = ps.enter_context(tc.tile_pool(name="dsb", bufs=2))
                    dzp = ps.enter_context(tc.tile_pool(name="dz", bufs=1))
                    vtp = ps.enter_context(tc.tile_pool(name="vt", bufs=2))
                    gtp = ps.enter_context(tc.tile_pool(name="gt", bufs=2))
                    smp = ps.enter_context(tc.tile_pool(name="sm", bufs=1))

                    # Software-pipelined emission: tile m's gather-consume
                    # (reduce/epilogue) is emitted AFTER tile m+1's top-k so
                    # the DVE works on tile m+1 while the Pool engine
                    # generates tile m's gather descriptors.
                    gsave = {}

                    def emit_front(m):
                        mc = slice(m * 128, (m + 1) * 128)
                        pd = pdp.tile([128, N], F32, space="PSUM", tag="pd",
                                      name=f"pd{li}_{m}")
                        for j in range(JC):
                            jc = slice(j * 512, (j + 1) * 512)
                            nc.tensor.matmul(
                                pd[:, jc], xT[:, mc], xT[:, jc],
                                start=True, stop=False,
                            )
                            nc.tensor.matmul(
                                pd[:, jc], ones_row[:, mc],
                                halfx2neg[:, jc],
                                start=False, stop=True,
                            )
                        dsb = dsbp.tile([128, N], F32, tag="dsb",
                                        name=f"dsb{li}_{m}")
                        nc.scalar.copy(out=dsb[:], in_=pd[:, :])

                        v = vtp.tile([128, 24], F32, tag="v", name=f"v{li}_{m}")
                        ix = vtp.tile([128, 24], U32, tag="ix", name=f"ix{li}_{m}")
                        dz = dzp.tile([128, N], F32, tag="dz", name=f"dz{li}_{m}")
                        nc.vector.max(out=v[:, 0:8], in_=dsb[:])
                        nc.vector.max_index(
                            out=ix[:, 0:8], in_max=v[:, 0:8], in_values=dsb[:]
                        )
                        nc.vector.match_replace(
                            out=dz[:], in_to_replace=v[:, 0:8], in_values=dsb[:],
                            imm_value=NEG_BIG,
                        )
                        nc.vector.max(out=v[:, 8:16], in_=dz[:])
                        nc.vector.max_index(
                            out=ix[:, 8:16], in_max=v[:, 8:16], in_values=dz[:]
                        )
                        nc.vector.match_replace(
                            out=dz[:], in_to_replace=v[:, 8:16], in_values=dz[:],
                            imm_value=NEG_BIG,
                        )
                        nc.vector.max(out=v[:, 16:24], in_=dz[:])
                        nc.vector.max_index(
                            out=ix[:, 16:24], in_max=v[:, 16:24], in_values=dz[:]
                        )

                        if debug and li == 0:
                            nc.sync.dma_start(out=dbg_ix[mc, :], in_=ix[:, :])
                            if m == 0:
                                nc.sync.dma_start(out=dbg_d0[:, :], in_=dsb[:, :])
                        g = gtp.tile([128, K * O], F32, tag="g", name=f"g{li}_{m}")
                        # Slot 0 is always the point itself (self-distance is
                        # the row max; a tie means an identical Z row), so it
                        # is a contiguous Z block -- fetch it with a plain
                        # HWDGE DMA instead of a Pool-engine indirect gather.
                        sg = nc.sync.dma_start(out=g[:, 0:O], in_=z_dram[li][mc, :])
                        for zw in z_writes:
                            add_dep_helper(
                                sg.ins, zw, sync=True,
                                reason="self-row read of z_dram (RAW)",
                            )
                        for t in range(1, K):
                            gi = nc.gpsimd.indirect_dma_start(
                                out=g[:, t * O : (t + 1) * O],
                                out_offset=None,
                                in_=z_dram[li][:, :],
                                in_offset=IndirectOffsetOnAxis(
                                    ap=ix[:, t : t + 1], axis=0
                                ),
                            )
                            if m == 0 and t == 1:
                                # All gathers sit behind this one on the same
                                # SWDGE FIFO queue, so one sync edge per conv
                                # orders every gather after the Z writes.
                                for zw in z_writes:
                                    add_dep_helper(
                                        gi.ins, zw, sync=True,
                                        reason="gather reads z_dram (RAW)",
                                    )
                        gsave[m] = g

                    def emit_back(m):
                        mc = slice(m * 128, (m + 1) * 128)
                        g = gsave.pop(m)
                        O_ = O
                        # view gathered [128, K, O] as [128, O, K]; reduce over K
                        ga = g[:, :]
                        gview = bass.AP(
                            ga.tensor, ga.offset, [ga.ap[0], [1, O_], [O_, K]]
                        )
                        mx = smp.tile([128, O_], F32, tag="mx", name=f"mx{li}_{m}")
                        nc.vector.reduce_max(out=mx[:], in_=gview, axis=X_AX)
                        if debug and li == 0 and m == 0:
                            nc.sync.dma_start(out=dbg_g0[:, :], in_=g[:, :])

                        y = smp.tile([128, O_], F32, tag="y", name=f"y{li}_{m}")
                        nc.vector.tensor_add(out=y[:], in0=mx[:], in1=csb[:, m, :])
                        yl = smp.tile([128, O_], F32, tag="yl", name=f"yl{li}_{m}")
                        nc.scalar.activation(
                            out=yl[:], in_=y[:], func=COPY, scale=0.2
                        )
                        nc.vector.tensor_tensor(
                            out=yl[:], in0=y[:], in1=yl[:], op=mybir.AluOpType.max
                        )

                        for tgt, ocs in conv_out[li]:
                            w = min(128, O_ - ocs)
                            ptr = ptrp.tile([128, 128], F32, space="PSUM",
                                            tag="ptr", name=f"ptr{li}_{m}_{ocs}")
                            nc.tensor.transpose(
                                out=ptr[0:w, :], in_=yl[:, ocs : ocs + w],
                                identity=identity[:],
                            )
                            nc.scalar.copy(out=tgt[:, mc], in_=ptr[0:w, :])

                    for m in range(T):
                        emit_front(m)
                        if m >= 1:
                            emit_back(m - 1)
                            if do_final and li == 3:
                                emit_final_tile(m - 1)
                    emit_back(T - 1)
                    if do_final and li == 3:
                        emit_final_tile(T - 1)

        if not do_final:
            dummy = persist.tile([1, 2 * 1024], F32)
            nc.vector.memset(dummy[:], 0.0)
            nc.sync.dma_start(out=out_t[:], in_=dummy[:, :])

        if debug:
            nc.sync.dma_start(out=dbg_x1[:], in_=x1T[:, :])
        if do_final:
            # ---- final epilogue: max+mean pool over N (W5 stage ran
            # interleaved with conv4 above) ----
            with ExitStack() as fs_stack:
                ptr2p = fs_stack.enter_context(
                    tc.tile_pool(name="ptr2", bufs=2, space="PSUM")
                )
                psp = fs_stack.enter_context(tc.tile_pool(name="ps", bufs=1, space="PSUM"))

                for c in range(8):
                    cc = slice(c * 128, (c + 1) * 128)
                    ptr2 = ptr2p.tile([128, 128], F32, space="PSUM", tag="ptr2")
                    nc.tensor.transpose(
                        out=ptr2[:], in_=maxacc[:, cc], identity=identity[:]
                    )
                    nc.vector.reduce_max(out=maxp[:, c : c + 1], in_=ptr2[:, :], axis=X_AX)
                psum_s = psp.tile([1, 1024], F32, space="PSUM", tag="psum_s")
                for h in range(2):
                    hc = slice(h * 512, (h + 1) * 512)
                    nc.tensor.matmul(
                        psum_s[:, hc], onescol[:, :], sumacc[:, hc],
                        start=True, stop=True,
                    )
                nc.scalar.activation(
                    out=avgp[:, :], in_=psum_s[:, :], func=COPY, scale=1.0 / N
                )

                outap = out_t[:]
                nc.sync.dma_start(
                    out=bass.AP(outap.tensor, 0, [[1, 128], [128, 8]]), in_=maxp[:, :]
                )
                nc.sync.dma_start(out=out_t[0:1, 1024:2048], in_=avgp[:, :])

    nc.finalize()
    return nc


def host_inputs(points, Ws, gs, bs, g5, b5, W5):
    """Host-side preprocessing -> per-core input maps (weights replicated)."""
    B = points.shape[0]
    shared = {}
    for li, (C, O) in enumerate(CONVS):
        W = np.asarray(Ws[li], np.float32)
        s = (np.asarray(gs[li], np.float32) / np.sqrt(np.float32(1.0 + EPS)))[:, None]
        A = (s * W[:, :C]).T.astype(np.float32)  # (C, O)
        Cm = (s * (W[:, C:] - W[:, :C])).T.astype(np.float32)  # (C, O)
        shared[f"acrhs{li}"] = np.ascontiguousarray(
            np.concatenate([A, Cm], axis=1), np.float32
        )
        shared[f"acb{li}"] = np.concatenate(
            [np.zeros((1, O), np.float32), np.asarray(bs[li], np.float32)[None, :]],
            axis=1,
        )
    s5 = (np.asarray(g5, np.float32) / np.sqrt(np.float32(1.0 + EPS)))[:, None]
    W5s = (s5 * np.asarray(W5, np.float32)).T.astype(np.float32)  # (512, 1024)
    ofs = 0
    for k, ck in enumerate([64, 64, 128, 128, 128]):
        shared[f"w5c{k}"] = np.ascontiguousarray(W5s[ofs : ofs + ck], np.float32)
        ofs += ck
    shared["b5row"] = np.asarray(b5, np.float32)[None, :]
    maps = []
    for b in range(B):
        m = dict(shared)
        m["pointsT"] = np.ascontiguousarray(
            np.asarray(points[b], np.float32).T, np.float32
        )
        maps.append(m)
    return maps


def _bust_stale_caches():
    # The libneuronxla NEFF cache key has been observed to collide across
    # different BIR payloads with identical HLO shapes, silently reusing a
    # stale NEFF.  A recompile is cheap insurance against wrong results.
    import shutil

    import glob
    import os

    dirs = [
        "/root/.neuron-compile-cache",
        "/tmp/no-user/neuroncc_compile_workdir",
        f"/tmp/neuron-compile-cache-uid{os.getuid()}",
    ] + glob.glob("/tmp/neuron-compile-cache-uid*")
    for d in dirs:
        shutil.rmtree(d, ignore_errors=True)


def kernel(points, W1, W2, W3, W4, W5, g1, g2, g3, g4, g5, b1, b2, b3, b4, b5):
    _bust_stale_caches()
    points = np.asarray(points, np.float32)
    B, N, _ = points.shape
    assert (B, N) == (8, 2048), (B, N)
    nc = build_program(N)
    in_maps = host_inputs(
        points, [W1, W2, W3, W4], [g1, g2, g3, g4], [b1, b2, b3, b4], g5, b5, W5
    )
    res = run_bass_kernel_spmd(nc, in_maps, list(range(8)))
    out = np.stack(
        [res.results[b]["out"].reshape(-1) for b in range(8)]
    ).astype(np.float32)
    return out



# revision 48
# speedup vs baseline: 1.0798x; 1.0798x over previous
"""DGCNN encoder Bass kernel for Trainium2 (8 NeuronCores, data-parallel over batch).

Algorithm notes (per core, one point cloud of N=2048 points):
  Each edge conv  y_i = max_{j in knn20(i)} bn_lrelu(W @ [x_j - x_i; x_i])
  is reformulated with A = W[:, :C], Cm = W[:, C:] - W[:, :C]:
      y_i = lrelu( s * (max_j (A x_j) + Cm x_i) + beta )       (s = gamma/sqrt(1+eps) > 0)
  so no per-edge features are ever materialized: Z = X A''^T (N x O) is computed
  once, the kNN top-20 row gather of Z happens via indirect DMA, and the max
  over neighbors commutes with the per-point additive term and the monotone
  bn_lrelu.  kNN ranking uses D = X X^T - 0.5*|x_j|^2 (row-rank-equivalent to
  the reference's -dist^2), top-20 per row via 3 rounds of DVE max8 /
  max_index / match_replace (exact).
"""

import sys

if "/opt/trn_rl_repo" not in sys.path:
    sys.path.insert(0, "/opt/trn_rl_repo")

from contextlib import ExitStack

import numpy as np

import concourse.bass as bass
import concourse.mybir as mybir
from concourse import bacc
from concourse.bass import IndirectOffsetOnAxis
from concourse.bass_utils import run_bass_kernel_spmd
from concourse.masks import make_identity
from concourse.tile import TileContext, add_dep_helper

EPS = 1e-5
K = 20
NEG_BIG = -3.0e38
F32 = mybir.dt.float32
F32R = mybir.dt.float32r
U32 = mybir.dt.uint32
X_AX = mybir.AxisListType.X
COPY = mybir.ActivationFunctionType.Copy
LRELU = mybir.ActivationFunctionType.Lrelu
SQUARE = mybir.ActivationFunctionType.Square
ADD = mybir.AluOpType.add
MAX = mybir.AluOpType.max


def _f(ap):
    """read stored (possibly f32r-rounded) activations as plain fp32 for the
    exact-arithmetic kNN ranking path."""
    return ap.bitcast(mybir.dt.float32)

# (C_in, O_out) per edge conv
CONVS = [(3, 64), (64, 64), (64, 128), (128, 256)]
# gather blocks: self + 19 neighbors
NBLK = 20
import os as _os

MERGED_GATHER = int(_os.environ.get("KM_MERGED_GATHER", "0"))
# 0 = all fp32; 1 = f32r everywhere (breaks kNN ranking: ~3.8 rel err);
# 2 = f32r on the value path only (weights / Z / W5) — ranking stays fp32
USE_F32R = int(_os.environ.get("KM_F32R", "0"))
USE_LRELU = int(_os.environ.get("KM_LRELU", "0"))
USE_POOL_ADD = int(_os.environ.get("KM_POOL_ADD", "0"))
# weights dtype (value path) and activations/points dtype (ranking path)
WDT = mybir.dt.float32r if USE_F32R >= 1 else mybir.dt.float32
XDT = mybir.dt.float32r if USE_F32R == 1 else mybir.dt.float32
PDT = WDT

_NONCE = None


def _build_nonce():
    """Distinct dummy-input size per build, keeping each build's HLO signature
    unique so stale-NEFF cache hits can't happen. Uses a monotonic counter
    persisted in /tmp so successive builds never collide."""
    global _NONCE
    if _NONCE is None:
        path = "/tmp/km_nonce_counter"
        try:
            n = int(open(path).read().strip())
        except Exception:
            n = 0
        try:
            with open(path, "w") as f:
                f.write(str(n + 1))
        except Exception:
            pass
        _NONCE = 8 + (n % 499)
    return _NONCE


def build_program(n_points=2048, max_conv=4, do_final=True, debug=False):
    """Build the single-core program (SPMD across 8 cores, batch-parallel)."""
    N = n_points
    T = N // 128  # row tiles
    JC = N // 512  # 512-wide column chunks of the distance matrix

    nc = bacc.Bacc(None, num_swdge_queues=4)

    pointsT = nc.declare_dram_parameter("pointsT", [3, N], XDT, isOutput=False)
    acrhs = []
    acb = []
    for li, (C, O) in enumerate(CONVS):
        cdt = F32 if li == 0 else PDT
        acrhs.append(
            nc.declare_dram_parameter(f"acrhs{li}", [C, 2 * O], cdt, isOutput=False)
        )
        acb.append(
            nc.declare_dram_parameter(f"acb{li}", [1, 2 * O], cdt, isOutput=False)
        )
    W5_SPLITS = [64, 64, 128, 128, 128]
    w5c = [
        nc.declare_dram_parameter(f"w5c{k}", [ck, 1024], PDT, isOutput=False)
        for k, ck in enumerate(W5_SPLITS)
    ]
    b5row = nc.declare_dram_parameter("b5row", [1, 1024], PDT, isOutput=False)
    ones_dram = nc.declare_dram_parameter("ones_row", [1, 128], PDT, isOutput=False)
    # The libneuronxla NEFF cache key does not cover the BIR payload riding in
    # the custom-call config, so two different programs with identical input
    # signatures collide and silently reuse a stale NEFF. A nonce-sized dummy
    # input makes every build's HLO signature unique.
    nonce = _build_nonce()
    nonce_in = nc.declare_dram_parameter("nonce", [1, nonce], F32, isOutput=False)
    out_t = nc.declare_dram_parameter("out", [1, 2 * 1024], F32, isOutput=True)
    if debug:
        dbg_ix = nc.declare_dram_parameter("dbg_ix", [N, 24], U32, isOutput=True)
        dbg_x1 = nc.declare_dram_parameter("dbg_x1", [64, N], F32, isOutputrue=True) if False else nc.declare_dram_parameter("dbg_x1", [64, N], F32, isOutput=True)
        dbg_hx = nc.declare_dram_parameter("dbg_hx", [1, N], F32, isOutput=True)
        dbg_z1 = nc.declare_dram_parameter("dbg_z1", [N, CONVS[0][1]], F32, isOutput=True)
        dbg_d0 = nc.declare_dram_parameter("dbg_d0", [128, N], F32, isOutput=True)
        dbg_g0 = nc.declare_dram_parameter("dbg_g0", [128, K * CONVS[0][1]], F32, isOutput=True)

    with ExitStack() as stack:
        tc = stack.enter_context(TileContext(nc))
        persist = stack.enter_context(tc.tile_pool(name="persist", bufs=1))
        dram = stack.enter_context(tc.tile_pool(name="dram", bufs=1, space="DRAM"))

        # ---- persistent SBUF state ----
        identity = persist.tile([128, 128], F32)
        make_identity(nc, identity[:])
        ones_row = persist.tile([1, 128], PDT)
        nc.sync.dma_start(out=ones_row[:], in_=ones_dram[:])
        ones_f32 = persist.tile([1, 128], F32, tag="ones_f32")
        nc.sync.dma_start(out=ones_f32[:], in_=ones_dram[:].bitcast(F32))
        onescol = persist.tile([128, 1], F32)
        nc.vector.memset(onescol[:], 1.0)
        halfx2neg = persist.tile([1, N], XDT)

        xt0 = persist.tile([3, N], XDT)
        nc.sync.dma_start(out=xt0[:], in_=pointsT[:])
        nonce_sb = persist.tile([1, nonce], F32, tag="nonce")
        nc.sync.dma_start(out=nonce_sb[:], in_=nonce_in[:])
        x1T = persist.tile([64, N], WDT)
        x2T = persist.tile([64, N], WDT)
        x3T = persist.tile([128, N], WDT)
        x4Ta = persist.tile([128, N], WDT)
        x4Tb = persist.tile([128, N], WDT)

        acrhs_sb = []
        acb_sb = []
        for li, (C, O) in enumerate(CONVS):
            cdt = F32 if li == 0 else PDT
            a = persist.tile([C, 2 * O], cdt, tag=f"acrhs{li}")
            nc.sync.dma_start(out=a[:], in_=acrhs[li][:])
            acrhs_sb.append(a)
            b = persist.tile([1, 2 * O], cdt, tag=f"acb{li}")
            nc.sync.dma_start(out=b[:], in_=acb[li][:])
            acb_sb.append(b)
        w5_sb = []
        for k, ck in enumerate(W5_SPLITS):
            w = persist.tile([ck, 1024], PDT, tag=f"w5c{k}")
            nc.sync.dma_start(out=w[:], in_=w5c[k][:])
            w5_sb.append(w)
        b5_sb = persist.tile([1, 1024], PDT)
        nc.sync.dma_start(out=b5_sb[:], in_=b5row[:])

        maxacc = persist.tile([128, 1024], F32)
        sumacc = persist.tile([128, 1024], F32)
        maxp = persist.tile([128, 8], F32)
        avgp = persist.tile([1, 1024], F32)

        z_dram = [
            dram.tile([N, O], F32, tag=f"z{li}", name=f"z{li}")
            for li, (_, O) in enumerate(CONVS)
        ]

        conv_in = [xt0, x1T, x2T, x3T]
        conv_out = [[(x1T, 0)], [(x2T, 0)], [(x3T, 0)], [(x4Ta, 0), (x4Tb, 128)]]

        # Final-stage pools are opened for the whole conv region so the W5
        # matmul + pooling accumulation for row-tile m can run interleaved
        # with conv4's pipeline as soon as x4T[:, mc] lands.
        pfp = stack.enter_context(tc.tile_pool(name="pf", bufs=1, space="PSUM"))
        fsp = stack.enter_context(tc.tile_pool(name="fs", bufs=1))
        if do_final:
            nc.vector.memset(maxacc[:], NEG_BIG)
            nc.vector.memset(sumacc[:], 0.0)
        xks = [x1T, x2T, x3T, x4Ta, x4Tb]

        def emit_final_tile(m):
            mc = slice(m * 128, (m + 1) * 128)
            pf = pfp.tile([128, 1024], F32, space="PSUM", tag="pf",
                          name=f"pf_{m}")
            for h in range(2):
                hc = slice(h * 512, (h + 1) * 512)
                for k, xk in enumerate(xks):
                    nc.tensor.matmul(
                        pf[:, hc], xk[:, mc], w5_sb[k][:, hc],
                        start=(k == 0), stop=False,
                    )
                nc.tensor.matmul(
                    pf[:, hc], ones_row[:, 0:128], b5_sb[:, hc],
                    start=False, stop=True,
                )
            fs = fsp.tile([128, 1024], F32, tag="fs", name=f"fs_{m}")
            if USE_LRELU:
                nc.scalar.activation(
                    out=fs[:], in_=pf[:, :], func=LRELU, alpha=0.2
                )
            else:
                nc.scalar.activation(
                    out=fs[:], in_=pf[:, :], func=COPY, scale=0.2
                )
                nc.vector.tensor_tensor(
                    out=fs[:], in0=pf[:, :], in1=fs[:], op=MAX
                )
            nc.vector.tensor_tensor(out=maxacc[:], in0=maxacc[:], in1=fs[:], op=MAX)
            nc.vector.tensor_add(out=sumacc[:], in0=sumacc[:], in1=fs[:])

        for li, (C, O) in enumerate(CONVS):
            if li >= max_conv:
                break
            xT = conv_in[li]
            with ExitStack() as cs:
                sqp = cs.enter_context(tc.tile_pool(name="sq", bufs=1))
                zstp = cs.enter_context(tc.tile_pool(name="zst", bufs=2))
                csbp = cs.enter_context(tc.tile_pool(name="csb", bufs=1))

                # ---- phase A: column norms -> halfx2neg = -0.5 * |x_j|^2 ----
                with tc.tile_pool(name="px2", bufs=1, space="PSUM") as px2p:
                    sq = sqp.tile([C, N], F32, tag="sq")
                    nc.scalar.activation(
                        out=sq[:], in_=xT[:, :].bitcast(F32), func=SQUARE
                    )
                    px2 = px2p.tile([1, N], F32, space="PSUM", tag="px2")
                    for j in range(JC):
                        jc = slice(j * 512, (j + 1) * 512)
                        nc.tensor.matmul(
                            px2[:, jc], onescol[0:C, :], sq[:, jc],
                            start=True, stop=True,
                        )
                    nc.scalar.activation(
                        out=halfx2neg[:, :], in_=px2[:, :], func=COPY, scale=-0.5
                    )
                    if debug and li == 0:
                        nc.sync.dma_start(out=dbg_hx[:], in_=halfx2neg[:, :])

                # ---- phase B: Z = X A''^T + 0*beta -> DRAM ; c = X Cm''^T + beta ----
                z_writes = []
                csb = csbp.tile([128, T, O], F32, tag="csb")
                with tc.tile_pool(name="pzc", bufs=2, space="PSUM") as pzcp:
                    for m in range(T):
                        mc = slice(m * 128, (m + 1) * 128)
                        pzc = pzcp.tile([128, 2 * O], F32, space="PSUM", tag="pzc")
                        nc.tensor.matmul(
                            pzc[:, :], xT[:, mc], acrhs_sb[li][:, :],
                            start=True, stop=False,
                        )
                        nc.tensor.matmul(
                            pzc[:, :],
                            (ones_f32 if li == 0 else ones_row)[:, 0:128],
                            acb_sb[li][:, :],
                            start=False, stop=True,
                        )
                        zst = zstp.tile([128, O], F32, tag="zst")
                        nc.scalar.copy(out=zst[:], in_=pzc[:, 0:O])
                        nc.scalar.copy(out=csb[:, m, :], in_=pzc[:, O : 2 * O])
                        zw = nc.sync.dma_start(out=z_dram[li][mc, :], in_=zst[:])
                        z_writes.append(zw.ins)
                        if debug and li == 0:
                            nc.sync.dma_start(out=dbg_z1[mc, :], in_=zst[:])

                # ---- phase C: per row-tile D, top-20, gather, reduce ----
                with ExitStack() as ps:
                    pdp = ps.enter_context(
                        tc.tile_pool(name="pd", bufs=1, space="PSUM")
                    )
                    ptrp = ps.enter_context(
                        tc.tile_pool(name="ptr", bufs=2, space="PSUM")
                    )
                    depth = int(_os.environ.get("KM_DEPTH", "3")) if li < 3 else 1
                    dsbp = ps.enter_context(tc.tile_pool(name="dsb", bufs=int(_os.environ.get("KM_DSB", "2"))))
                    dzp = ps.enter_context(tc.tile_pool(name="dz", bufs=1))
                    vtp = ps.enter_context(tc.tile_pool(name="vt", bufs=depth + 1))
                    gtp = ps.enter_context(tc.tile_pool(name="gt", bufs=depth + 1))
                    smp = ps.enter_context(tc.tile_pool(name="sm", bufs=2))

                    # Software-pipelined emission: tile m's gather-consume
                    # (reduce/epilogue) is emitted AFTER tile m+1's top-k so
                    # the DVE works on tile m+1 while the Pool engine
                    # generates tile m's gather descriptors.
                    gsave = {}

                    def emit_front(m):
                        mc = slice(m * 128, (m + 1) * 128)
                        pd = pdp.tile([128, N], F32, space="PSUM", tag="pd",
                                      name=f"pd{li}_{m}")
                        for j in range(JC):
                            jc = slice(j * 512, (j + 1) * 512)
                            nc.tensor.matmul(
                                pd[:, jc], _f(xT[:, mc]), _f(xT[:, jc]),
                                start=True, stop=False,
                            )
                            nc.tensor.matmul(
                                pd[:, jc], ones_f32[:, 0:128],
                                halfx2neg[:, jc],
                                start=False, stop=True,
                            )
                        dsb = dsbp.tile([128, N], F32, tag="dsb",
                                        name=f"dsb{li}_{m}")
                        nc.scalar.copy(out=dsb[:], in_=pd[:, :])

                        v = vtp.tile([128, 24], F32, tag="v", name=f"v{li}_{m}")
                        ix = vtp.tile([128, 24], U32, tag="ix", name=f"ix{li}_{m}")
                        dz = dzp.tile([128, N], F32, tag="dz", name=f"dz{li}_{m}")
                        nc.vector.max(out=v[:, 0:8], in_=dsb[:])
                        nc.vector.max_index(
                            out=ix[:, 0:8], in_max=v[:, 0:8], in_values=dsb[:]
                        )
                        nc.vector.match_replace(
                            out=dz[:], in_to_replace=v[:, 0:8], in_values=dsb[:],
                            imm_value=NEG_BIG,
                        )
                        nc.vector.max(out=v[:, 8:16], in_=dz[:])
                        nc.vector.max_index(
                            out=ix[:, 8:16], in_max=v[:, 8:16], in_values=dz[:]
                        )
                        nc.vector.match_replace(
                            out=dz[:], in_to_replace=v[:, 8:16], in_values=dz[:],
                            imm_value=NEG_BIG,
                        )
                        nc.vector.max(out=v[:, 16:24], in_=dz[:])
                        nc.vector.max_index(
                            out=ix[:, 16:24], in_max=v[:, 16:24], in_values=dz[:]
                        )

                        if debug and li == 0:
                            nc.sync.dma_start(out=dbg_ix[mc, :], in_=ix[:, :])
                            if m == 0:
                                nc.sync.dma_start(out=dbg_d0[:, :], in_=dsb[:, :])
                        g = gtp.tile([128, NBLK * O], F32, tag="g",
                                     name=f"g{li}_{m}")
                        # Slot 0 is always the point itself (self-distance is
                        # the row max; a tie means an identical Z row), so it
                        # is a contiguous Z block -- fetch it with a plain
                        # HWDGE DMA instead of a Pool-engine indirect gather.
                        sg = nc.sync.dma_start(out=g[:, 0:O], in_=z_dram[li][mc, :])
                        for zw in z_writes:
                            add_dep_helper(
                                sg.ins, zw, sync=True,
                                reason="self-row read of z_dram (RAW)",
                            )
                        # Neighbor rows in 3 merged indirect gathers (SWDGE
                        # cost is ~1us fixed per instruction, so merging 19
                        # single-index gathers is ~6x cheaper on Pool; 3
                        # instructions keep each under the 1024-descriptor
                        # SWDGE ring carveout).
                        gather_splits = (
                            [(t, t + 1) for t in range(1, K)]
                            if MERGED_GATHER == 0
                            else [(1, 8), (8, 14), (14, 20)]
                        )
                        for si, (lo, hi) in enumerate(gather_splits):
                            gi = nc.gpsimd.indirect_dma_start(
                                out=g[:, lo * O : hi * O],
                                out_offset=None,
                                in_=z_dram[li][:, :],
                                in_offset=IndirectOffsetOnAxis(
                                    ap=ix[:, lo:hi], axis=0
                                ),
                            )
                            if m == 0 and si == 0:
                                # All gathers sit behind this one on the same
                                # SWDGE FIFO queue, so one sync edge per conv
                                # orders every gather after the Z writes.
                                for zw in z_writes:
                                    add_dep_helper(
                                        gi.ins, zw, sync=True,
                                        reason="gather reads z_dram (RAW)",
                                    )
                        gsave[m] = g

                    def emit_back(m):
                        mc = slice(m * 128, (m + 1) * 128)
                        g = gsave.pop(m)
                        O_ = O
                        # view gathered [128, NBLK, O] as [128, O, NBLK]
                        ga = g[:, :]
                        gview = bass.AP(
                            ga.tensor, ga.offset, [ga.ap[0], [1, O_], [O_, NBLK]]
                        )
                        mx = smp.tile([128, O_], F32, tag="mx", name=f"mx{li}_{m}")
                        nc.vector.reduce_max(out=mx[:], in_=gview, axis=X_AX)
                        if debug and li == 0 and m == 0:
                            nc.sync.dma_start(out=dbg_g0[:, :], in_=g[:, :])

                        y = smp.tile([128, O_], F32, tag="y", name=f"y{li}_{m}")
                        eng = nc.gpsimd if USE_POOL_ADD else nc.vector
                        eng.tensor_tensor(
                            out=y[:], in0=mx[:], in1=csb[:, m, :], op=ADD
                        )
                        yl = smp.tile([128, O_], F32, tag="yl", name=f"yl{li}_{m}")
                        if USE_LRELU:
                            nc.scalar.activation(
                                out=yl[:], in_=y[:], func=LRELU, alpha=0.2
                            )
                        else:
                            nc.scalar.activation(
                                out=yl[:], in_=y[:], func=COPY, scale=0.2
                            )
                            nc.vector.tensor_tensor(
                                out=yl[:], in0=y[:], in1=yl[:], op=MAX
                            )

                        for tgt, ocs in conv_out[li]:
                            w = min(128, O_ - ocs)
                            ptr = ptrp.tile([128, 128], F32, space="PSUM",
                                            tag="ptr", name=f"ptr{li}_{m}_{ocs}")
                            nc.tensor.transpose(
                                out=ptr[0:w, :], in_=yl[:, ocs : ocs + w],
                                identity=identity[:],
                            )
                            # write rounded to fp32r: these tiles feed fp32r
                            # matmuls (the BIR verifier requires producers of
                            # fp32r matmul inputs to round on write)
                            nc.scalar.copy(out=tgt[:, mc], in_=ptr[0:w, :])

                    for m in range(T):
                        emit_front(m)
                        if m >= depth:
                            emit_back(m - depth)
                            if do_final and li == 3:
                                emit_final_tile(m - depth)
                    for m in range(T - depth, T):
                        emit_back(m)
                        if do_final and li == 3:
                            emit_final_tile(m)

        if not do_final:
            dummy = persist.tile([1, 2 * 1024], F32)
            nc.vector.memset(dummy[:], 0.0)
            nc.sync.dma_start(out=out_t[:], in_=dummy[:, :])

        if debug:
            nc.sync.dma_start(out=dbg_x1[:], in_=x1T[:, :])
        if do_final:
            # ---- final epilogue: max+mean pool over N (W5 stage ran
            # interleaved with conv4 above) ----
            with ExitStack() as fs_stack:
                ptr2p = fs_stack.enter_context(
                    tc.tile_pool(name="ptr2", bufs=2, space="PSUM")
                )
                psp = fs_stack.enter_context(tc.tile_pool(name="ps", bufs=1, space="PSUM"))

                for c in range(8):
                    cc = slice(c * 128, (c + 1) * 128)
                    ptr2 = ptr2p.tile([128, 128], F32, space="PSUM", tag="ptr2")
                    nc.tensor.transpose(
                        out=ptr2[:], in_=maxacc[:, cc], identity=identity[:]
                    )
                    nc.vector.reduce_max(out=maxp[:, c : c + 1], in_=ptr2[:, :], axis=X_AX)
                psum_s = psp.tile([1, 1024], F32, space="PSUM", tag="psum_s")
                for h in range(2):
                    hc = slice(h * 512, (h + 1) * 512)
                    nc.tensor.matmul(
                        psum_s[:, hc], onescol[:, :], sumacc[:, hc],
                        start=True, stop=True,
                    )
                nc.scalar.activation(
                    out=avgp[:, :], in_=psum_s[:, :], func=COPY, scale=1.0 / N
                )

                outap = out_t[:]
                nc.sync.dma_start(
                    out=bass.AP(outap.tensor, 0, [[1, 128], [128, 8]]), in_=maxp[:, :]
                )
                nc.sync.dma_start(out=out_t[0:1, 1024:2048], in_=avgp[:, :])

    nc.finalize()
    return nc


def host_inputs(points, Ws, gs, bs, g5, b5, W5):
    """Host-side preprocessing -> per-core input maps (weights replicated)."""
    B = points.shape[0]
    shared = {}
    for li, (C, O) in enumerate(CONVS):
        W = np.asarray(Ws[li], np.float32)
        s = (np.asarray(gs[li], np.float32) / np.sqrt(np.float32(1.0 + EPS)))[:, None]
        A = (s * W[:, :C]).T.astype(np.float32)  # (C, O)
        Cm = (s * (W[:, C:] - W[:, :C])).T.astype(np.float32)  # (C, O)
        shared[f"acrhs{li}"] = np.ascontiguousarray(
            np.concatenate([A, Cm], axis=1), np.float32
        )
        shared[f"acb{li}"] = np.concatenate(
            [np.zeros((1, O), np.float32), np.asarray(bs[li], np.float32)[None, :]],
            axis=1,
        )
    s5 = (np.asarray(g5, np.float32) / np.sqrt(np.float32(1.0 + EPS)))[:, None]
    W5s = (s5 * np.asarray(W5, np.float32)).T.astype(np.float32)  # (512, 1024)
    ofs = 0
    for k, ck in enumerate([64, 64, 128, 128, 128]):
        shared[f"w5c{k}"] = np.ascontiguousarray(W5s[ofs : ofs + ck], np.float32)
        ofs += ck
    shared["b5row"] = np.asarray(b5, np.float32)[None, :]
    shared["ones_row"] = np.ones((1, 128), np.float32)
    shared["nonce"] = np.zeros((1, _build_nonce()), np.float32)
    maps = []
    for b in range(B):
        m = dict(shared)
        m["pointsT"] = np.ascontiguousarray(
            np.asarray(points[b], np.float32).T, np.float32
        )
        maps.append(m)
    return maps


def _bust_stale_caches():
    # The libneuronxla NEFF cache key has been observed to collide across
    # different BIR payloads with identical HLO shapes, silently reusing a
    # stale NEFF.  A recompile is cheap insurance against wrong results.
    import shutil

    import glob
    import os

    dirs = [
        "/root/.neuron-compile-cache",
        "/tmp/no-user/neuroncc_compile_workdir",
        f"/tmp/neuron-compile-cache-uid{os.getuid()}",
    ] + glob.glob("/tmp/neuron-compile-cache-uid*")
    for d in dirs:
        shutil.rmtree(d, ignore_errors=True)


def kernel(points, W1, W2, W3, W4, W5, g1, g2, g3, g4, g5, b1, b2, b3, b4, b5):
    _bust_stale_caches()
    points = np.asarray(points, np.float32)
    B, N, _ = points.shape
    assert (B, N) == (8, 2048), (B, N)
    nc = build_program(N)
    in_maps = host_inputs(
        points, [W1, W2, W3, W4], [g1, g2, g3, g4], [b1, b2, b3, b4], g5, b5, W5
    )
    res = run_bass_kernel_spmd(nc, in_maps, list(range(8)))
    out = np.stack(
        [res.results[b]["out"].reshape(-1) for b in range(8)]
    ).astype(np.float32)
    return out

